# revision 37
# baseline (speedup 1.0000x reference)
"""Trainium2 Bass kernel for a Swin-style local-window ViT block.

Problem (hardcoded): x (4, 256, 256, 96) fp32, 8x8 windows, 3 heads (hd=32),
LN -> window attention (+rel-pos bias) -> proj -> residual -> LN -> MLP(4x,
gelu) -> residual.

Sharding: data-parallel. (B*H)=1024 image rows are split into 8 slabs of 128
rows; each slab holds 512 complete 8x8 windows, so the 8 cores are fully
independent (weights replicated).

v2 design notes (vs the first working version):
  - One DMA per window-pair ([[768,2],[24576,8],[1,768]]) -> 4 loads + 4
    stores per 512-token group instead of 16, dispatched on SP (loads) and
    SP/DVE/Pool (stores).
  - x2 (post-attention residual) stays resident in SBUF (96 KB/partition),
    removing the DRAM round trip entirely.
  - Scores matmuls merge the two windows of a pair (12 instead of 24 per
    group); the rel-pos bias is ADDED in PSUM by a second accumulating
    matmul (stationary = bias^T, moving = identity) with -30 as the
    cross-window mask, replacing the big DVE exp-bias multiply.
  - attn@V merges pairs too (12 matmuls) and the softmax denominator is
    produced by an appended ones column (h_t carries a constant-1 channel
    and the V weight block an extra unit column per head), so no separate
    row-sum matmuls.
  - bn_aggr is replaced by 5 tiny strided DVE ops that merge bn_stats'
    two half-triples exactly.
  - PSUM->SBUF copies ride the scalar engine so PE consumers see
    single-engine dependencies.
"""

import sys

sys.path.insert(0, "/opt/trn_rl_repo")

import numpy as np

import concourse.bass as bass
import concourse.bacc as bacc
import concourse.tile as tile
from concourse import mybir
from concourse import bass_utils

F32 = mybir.dt.float32
BF16 = mybir.dt.bfloat16
AF = mybir.ActivationFunctionType
ALU = mybir.AluOpType

B, H, W, C = 4, 256, 256, 96
WIN = 8
HEADS = 3
HD = 32
SCALE = HD ** -0.5
HID = 4 * C

NCORES = 8
ROWS = (B * H) // NCORES          # 128 image rows per core
NGROUPS = 64                      # groups of 512 tokens (8 windows) per core
SUPER = 8                         # groups per rstd batch
EPS = 1e-5
NEG = -30.0                       # additive mask for cross-window keys


def _rel_pos_index():
    coords = np.stack(np.meshgrid(np.arange(WIN), np.arange(WIN), indexing="ij")).reshape(2, -1)
    rel = coords[:, :, None] - coords[:, None, :]
    rel = rel.transpose(1, 2, 0).astype(np.int64)
    rel[:, :, 0] += WIN - 1
    rel[:, :, 1] += WIN - 1
    rel[:, :, 0] *= 2 * WIN - 1
    return rel.sum(-1)  # (64, 64)


REL_IDX = _rel_pos_index()

_CACHE = {}


def _pair_dram_ap(handle, g, wp):
    """DRAM AP for window-pair wp of group g: [8(r), 1536].

    Element order matches SBUF [128, 96] token-major with partition
    p = r*16 + par*8 + c (r-major token order), then 96 channels.
    The balanced refinement is [r 8][p-chunk 16][ch 96] = 3 dims.
    """
    wr, gc = divmod(g, 4)
    r0 = wr * WIN
    c0 = gc * 64 + wp * 16
    off = (r0 * W + c0) * C
    return bass.AP(
        tensor=handle,
        offset=off,
        ap=[[W * C, WIN], [1, 16 * C]],
    )


def _tok_idx(p):
    """Token index within its window for partition p (r-major order)."""
    return (p // 16) * 8 + (p % 8)


def _tok_par(p):
    """Which window of the pair (0/1) partition p belongs to."""
    return (p // 8) % 2


def _merge_halves(nc, st_tensor, st_off, st_pap, n, var_out, msum_out, scr):
    """Exact merge of bn_stats' two half-triples for n windows at once.

    Stats at st_off: [n, 6] per partition = (cnt, mean, M2) x 2 halves per
    window. Produces var_out [128, n] = 96*variance (M21 + M22 +
    24*(m1-m2)^2) and msum_out [128, n] = m1 + m2 (mean = msum/2).
    """
    def f(k):
        return bass.AP(tensor=st_tensor, offset=st_off + k,
                       ap=[st_pap, [6, n], [1, 1]])
    m1, M21, m2, M22 = f(1), f(2), f(4), f(5)
    dm = scr[:, 0:n]
    d2 = scr[:, n:2 * n]
    t = scr[:, 2 * n:3 * n]
    nc.vector.tensor_tensor(out=dm, in0=m1, in1=m2, op=ALU.subtract)
    nc.vector.tensor_tensor(out=msum_out, in0=m1, in1=m2, op=ALU.add)
    nc.vector.tensor_tensor(out=d2, in0=dm, in1=dm, op=ALU.mult)
    nc.vector.tensor_tensor(out=t, in0=M21, in1=M22, op=ALU.add)
    nc.vector.scalar_tensor_tensor(out=var_out, in0=d2, scalar=24.0, in1=t,
                                   op0=ALU.mult, op1=ALU.add)


def _build_program():
    nc = bacc.Bacc("TRN2", target_bir_lowering=False, debug=False)

    x_h = nc.dram_tensor("x", [ROWS, W, C], F32, kind="ExternalInput")
    out_h = nc.dram_tensor("out", [ROWS, W, C], F32, kind="ExternalOutput")

    wqk_h = nc.dram_tensor("wqk", [C, 2 * C], BF16, kind="ExternalInput")
    wv_h = nc.dram_tensor("wv", [C + 1, 99], BF16, kind="ExternalInput")
    wp_h = nc.dram_tensor("wproj", [C, C], BF16, kind="ExternalInput")
    w1_h = nc.dram_tensor("w1", [C, HID], BF16, kind="ExternalInput")
    w2_h = nc.dram_tensor("w2", [3, 128, C], BF16, kind="ExternalInput")
    logb_h = nc.dram_tensor("logbT", [128, 3, 128], BF16, kind="ExternalInput")
    ident_h = nc.dram_tensor("ident", [128, 128], BF16, kind="ExternalInput")
    identrep_h = nc.dram_tensor("identrep", [128, 512], BF16, kind="ExternalInput")

    with tile.TileContext(nc) as tc:
        with tc.tile_pool(name="const", bufs=1) as cpool:
            wqk = cpool.tile([C, 2 * C], BF16)
            nc.sync.dma_start(out=wqk, in_=wqk_h.ap())
            wv = cpool.tile([C + 1, 99], BF16)
            nc.sync.dma_start(out=wv, in_=wv_h.ap())
            wproj = cpool.tile([C, C], BF16)
            nc.sync.dma_start(out=wproj, in_=wp_h.ap())
            w1 = cpool.tile([C, HID], BF16)
            nc.sync.dma_start(out=w1, in_=w1_h.ap())
            w2 = cpool.tile([128, 3, C], BF16)
            nc.sync.dma_start(out=w2, in_=w2_h.ap().rearrange("c p f -> p c f"))
            logb = cpool.tile([128, 3, 128], BF16)
            nc.sync.dma_start(out=logb, in_=logb_h.ap())
            ident = cpool.tile([128, 128], BF16)
            nc.sync.dma_start(out=ident, in_=ident_h.ap())
            identrep = cpool.tile([128, 512], BF16)
            nc.sync.dma_start(out=identrep, in_=identrep_h.ap())
            epsb = cpool.tile([128, 1], F32)
            nc.vector.memset(epsb, EPS)

            x2_all = cpool.tile([128, NGROUPS, 4, C], BF16)
            st2_all = cpool.tile([128, NGROUPS, 4, 6], F32)
            var2_all = cpool.tile([128, NGROUPS * 4], F32)
            rstd2_all = cpool.tile([128, NGROUPS * 4], F32)
            nmr2_all = cpool.tile([128, NGROUPS * 4], F32)
            scr2_all = cpool.tile([128, NGROUPS * 12], F32)

            # Warm-up: make PE/ACT observe each const-load DMA semaphore via a
            # tiny op, so real instructions never need two sync waits (this
            # walrus build supports at most one per instruction).
            with tc.tile_pool(name="warm", bufs=1, space="PSUM") as wps:
                wp_t = wps.tile([1, 8], F32)
                def _tiny(t):
                    base = t[:]
                    return bass.AP(tensor=base.tensor, offset=base.offset,
                                   ap=[[base.ap[0][0], 1], [1, 1]])
                for ci, cst in enumerate((wqk, wv, wproj, w1, w2, logb, ident,
                                          identrep)):
                    nc.tensor.matmul(wp_t[0:1, ci:ci + 1], _tiny(cst),
                                     _tiny(cst), start=True, stop=True)
                wd = cpool.tile([1, 1], BF16)
                nc.scalar.activation(out=wd, in_=_tiny(logb), func=AF.Copy)

            # ---------------- Phase A: attention ----------------
            with (
                tc.tile_pool(name="xin", bufs=SUPER + 5) as xpool,
                tc.tile_pool(name="stat", bufs=SUPER + 2) as stpool,
                tc.tile_pool(name="vs", bufs=2) as vspool,
                tc.tile_pool(name="sba", bufs=2) as sba,
                tc.tile_pool(name="psW", bufs=5, space="PSUM") as psW,
                tc.tile_pool(name="psS", bufs=1, space="PSUM") as psS,
            ):
                def do_front(g, x_t, rstd1, nmr_all, gi):
                    # LN1 apply: h = x*rstd + nmr (2 windows on ACT, 2 on DVE)
                    h_t = sba.tile([128, 4, C + 1], BF16, tag="h")
                    for j in range(4):
                        if j % 2 == 0:
                            nc.scalar.activation(
                                out=h_t[:, j, 0:C], in_=x_t[:, j, :],
                                func=AF.Identity,
                                scale=rstd1[:, gi * 4 + j:gi * 4 + j + 1],
                                bias=nmr_all[:, gi * 4 + j:gi * 4 + j + 1])
                        else:
                            nc.vector.tensor_scalar(
                                out=h_t[:, j, 0:C], in0=x_t[:, j, :],
                                scalar1=rstd1[:, gi * 4 + j:gi * 4 + j + 1],
                                scalar2=nmr_all[:, gi * 4 + j:gi * 4 + j + 1],
                                op0=ALU.mult, op1=ALU.add)
                    # column C stays 1.0: the two ring buffers for tag "h"
                    # were pre-initialized below and nothing else writes it

                    hT_ps = psW.tile([C + 1, 512], BF16, tag="w")
                    for j in range(4):
                        nc.tensor.transpose(hT_ps[:, j * 128:(j + 1) * 128],
                                            h_t[:, j, :], ident[:])
                    hT = sba.tile([C + 1, 512], BF16, tag="hT")
                    nc.vector.tensor_copy(out=hT, in_=hT_ps[:])

                    qT_ps = psW.tile([C, 512], F32, tag="w")
                    nc.tensor.matmul(qT_ps[:], wqk[:, 0:C], hT[0:C, :],
                                     start=True, stop=True)
                    kT_ps = psW.tile([C, 512], F32, tag="w")
                    nc.tensor.matmul(kT_ps[:], wqk[:, C:2 * C], hT[0:C, :],
                                     start=True, stop=True)
                    v_ps = psW.tile([128, 4, 99], F32, tag="w")
                    for j in range(4):
                        nc.tensor.matmul(v_ps[:, j, :],
                                         hT[:, j * 128:(j + 1) * 128],
                                         wv[:], start=True, stop=True)
                    qT = sba.tile([C, 512], BF16, tag="qT")
                    nc.scalar.activation(out=qT, in_=qT_ps[:], func=AF.Copy)
                    kT = sba.tile([C, 512], BF16, tag="kT")
                    nc.scalar.activation(out=kT, in_=kT_ps[:], func=AF.Copy)
                    v_t = sba.tile([128, 4, 99], BF16, tag="v")
                    nc.vector.tensor_copy(out=v_t, in_=v_ps[:])
                    return dict(g=g, x_t=x_t, qT=qT, kT=kT, v_t=v_t)

                def do_mid(ctx):
                    qT, kT = ctx["qT"], ctx["kT"]
                    sc_ps = psS.tile([128, 3, 512], F32, tag="sc")
                    for wp in range(4):
                        for hh in range(HEADS):
                            nc.tensor.matmul(
                                sc_ps[:, hh, wp * 128:(wp + 1) * 128],
                                kT[hh * HD:(hh + 1) * HD, wp * 128:(wp + 1) * 128],
                                qT[hh * HD:(hh + 1) * HD, wp * 128:(wp + 1) * 128],
                                start=True, stop=False,
                                tile_position=(hh * HD, 0),
                            )
                    for hh in range(HEADS):
                        nc.tensor.matmul(
                            sc_ps[:, hh, :], logb[:, hh, :], identrep[:],
                            start=False, stop=True,
                        )
                    E_t = sba.tile([128, 3, 512], BF16, tag="E")
                    nc.scalar.activation(out=E_t, in_=sc_ps[:], func=AF.Exp)
                    ctx["E_t"] = E_t

                def do_back1(ctx):
                    E_t, v_t = ctx["E_t"], ctx["v_t"]
                    o_ps = psW.tile([128, 4, 99], F32, tag="w")
                    for wp in range(4):
                        for hh in range(HEADS):
                            nc.tensor.matmul(
                                o_ps[:, wp, hh * 33:(hh + 1) * 33],
                                E_t[:, hh, wp * 128:(wp + 1) * 128],
                                v_t[:, wp, hh * 33:(hh + 1) * 33],
                                start=True, stop=True,
                            )
                    rs = stpool.tile([128, 12], F32, tag="rs")
                    s_ap = bass.AP(tensor=o_ps.tensor,
                                   offset=o_ps[:].offset + 32,
                                   ap=[o_ps[:].ap[0], [99, 4], [33, 3], [1, 1]])
                    nc.vector.reciprocal(out=rs, in_=s_ap)
                    o_t = sba.tile([128, 4, C], BF16, tag="o")
                    o_src = bass.AP(tensor=o_ps.tensor, offset=o_ps[:].offset,
                                    ap=[o_ps[:].ap[0], [99, 4], [33, 3], [1, HD]])
                    rs_b = bass.AP(tensor=rs.tensor, offset=rs[:].offset,
                                   ap=[rs[:].ap[0], [3, 4], [1, 3], [0, HD]])
                    o_dst = o_t[:].rearrange("p a (h d) -> p a h d", h=HEADS)
                    nc.vector.tensor_tensor(out=o_dst, in0=o_src, in1=rs_b,
                                            op=ALU.mult)
                    ctx["o_t"] = o_t

                def do_back2(ctx):
                    g, o_t, x_t = ctx["g"], ctx["o_t"], ctx["x_t"]
                    oT_ps = psW.tile([C, 512], BF16, tag="w")
                    for j in range(4):
                        nc.tensor.transpose(oT_ps[:, j * 128:(j + 1) * 128],
                                            o_t[:, j, :], ident[:])
                    oT = sba.tile([C, 512], BF16, tag="oT")
                    nc.vector.tensor_copy(out=oT, in_=oT_ps[:])

                    att_ps = psW.tile([128, 4, C], F32, tag="w")
                    for j in range(4):
                        nc.tensor.matmul(att_ps[:, j, :],
                                         oT[:, j * 128:(j + 1) * 128],
                                         wproj[:], start=True, stop=True)
                    nc.vector.scalar_tensor_tensor(
                        out=x2_all[:, g], in0=att_ps[:], scalar=1.0,
                        in1=x_t[:], op0=ALU.mult, op1=ALU.add)
                    for j in range(4):
                        nc.vector.bn_stats(out=st2_all[:, g, j, :],
                                           in_=x2_all[:, g, j, :])

                # pre-initialize the constant-1 column in the two "h" ring
                # buffers (LN applies never touch column C, transposes read it)
                for _ in range(2):
                    h_init = sba.tile([128, 4, C + 1], BF16, tag="h")
                    nc.vector.memset(h_init[:, :, C:C + 1], 1.0)

                inflight = []
                for sb in range(NGROUPS // SUPER):
                    x_ts = []
                    stS = stpool.tile([128, SUPER, 4, 6], F32, tag="st")
                    scr = stpool.tile([128, SUPER * 12], F32, tag="scr")
                    var1 = vspool.tile([128, SUPER * 4], F32, tag="var1")
                    msum = vspool.tile([128, SUPER * 4], F32, tag="msum")
                    for gi in range(SUPER):
                        g = sb * SUPER + gi
                        x_t = xpool.tile([128, 4, C], F32, tag="x")
                        for wp in range(4):
                            nc.sync.dma_start(out=x_t[:, wp, :],
                                              in_=_pair_dram_ap(x_h, g, wp))
                        for j in range(4):
                            nc.vector.bn_stats(out=stS[:, gi, j, :],
                                               in_=x_t[:, j, :])
                        x_ts.append(x_t)
                    # one batched merge for all SUPER*4 windows
                    _merge_halves(nc, stS.tensor, stS[:].offset, stS[:].ap[0],
                                  SUPER * 4, var1[:], msum[:], scr)
                    # batched rstd = exp(-0.5 * ln(var96/96 + eps))
                    lnv = vspool.tile([128, SUPER * 4], F32, tag="lnv")
                    nc.scalar.activation(out=lnv, in_=var1[:], func=AF.Ln,
                                         bias=epsb[:], scale=1.0 / 96.0)
                    rstd1 = vspool.tile([128, SUPER * 4], F32, tag="rstd1")
                    nc.scalar.activation(out=rstd1, in_=lnv[:], func=AF.Exp,
                                         scale=-0.5)
                    # nmr = -mean*rstd = -0.5*msum*rstd (batched)
                    nmr_all = vspool.tile([128, SUPER * 4], F32, tag="nmr")
                    nc.vector.scalar_tensor_tensor(
                        out=nmr_all, in0=msum[:], scalar=-0.5,
                        in1=rstd1[:], op0=ALU.mult, op1=ALU.mult)

                    for gi in range(SUPER):
                        g = sb * SUPER + gi
                        inflight.append(do_front(g, x_ts[gi], rstd1,
                                                 nmr_all, gi))
                        if len(inflight) >= 2:
                            do_mid(inflight[-2])
                        if len(inflight) >= 3:
                            do_back1(inflight[-3])
                        if len(inflight) >= 4:
                            do_back2(inflight.pop(0))
                # pipeline flush
                do_mid(inflight[-1])
                do_back1(inflight[-2])
                do_back2(inflight.pop(0))
                do_back1(inflight[-1])
                do_back2(inflight.pop(0))
                do_back2(inflight.pop(0))

                # batched LN2 stats merge + rstd for ALL groups at once
                # (still on the exp/ln table)
                _merge_halves(nc, st2_all.tensor, st2_all[:].offset,
                              st2_all[:].ap[0], NGROUPS * 4,
                              var2_all[:], nmr2_all[:], scr2_all)
                lnv2 = cpool.tile([128, NGROUPS * 4], F32)
                nc.scalar.activation(out=lnv2, in_=var2_all[:], func=AF.Ln,
                                     bias=epsb[:], scale=1.0 / 96.0)
                nc.scalar.activation(out=rstd2_all[:], in_=lnv2[:], func=AF.Exp,
                                     scale=-0.5)
                # nmr2 = -mean*rstd = -0.5*meansum*rstd
                nc.vector.scalar_tensor_tensor(
                    out=nmr2_all[:], in0=nmr2_all[:], scalar=-0.5,
                    in1=rstd2_all[:], op0=ALU.mult, op1=ALU.mult)

            # ---------------- Phase B: MLP ----------------
            with (
                tc.tile_pool(name="sbb", bufs=2) as sbb,
                tc.tile_pool(name="psT2", bufs=2, space="PSUM") as psT2,
                tc.tile_pool(name="psF", bufs=2, space="PSUM") as psF,
                tc.tile_pool(name="psG", bufs=4, space="PSUM") as psG,
            ):
                def b_front(g):
                    h2 = sbb.tile([128, 4, C], BF16, tag="h2")
                    for j in range(4):
                        nc.vector.tensor_scalar(
                            out=h2[:, j, :], in0=x2_all[:, g, j, :],
                            scalar1=rstd2_all[:, g * 4 + j:g * 4 + j + 1],
                            scalar2=nmr2_all[:, g * 4 + j:g * 4 + j + 1],
                            op0=ALU.mult, op1=ALU.add)
                    h2T_ps = psT2.tile([C, 512], BF16, tag="t2")
                    for j in range(4):
                        nc.tensor.transpose(h2T_ps[:, j * 128:(j + 1) * 128],
                                            h2[:, j, :], ident[:])
                    h2T = sbb.tile([C, 512], BF16, tag="h2T")
                    nc.vector.tensor_copy(out=h2T, in_=h2T_ps[:])
                    return dict(g=g, h2T=h2T)

                def b_mid(ctx):
                    g1 = sbb.tile([128, 3, 512], BF16, tag="g1s")
                    for ch in range(3):
                        g1_ps = psG.tile([128, 512], F32, tag="g1")
                        nc.tensor.matmul(g1_ps[:],
                                         w1[:, ch * 128:(ch + 1) * 128],
                                         ctx["h2T"][:], start=True, stop=True)
                        nc.scalar.activation(out=g1[:, ch, :], in_=g1_ps[:],
                                             func=AF.Gelu)
                    ctx["g1"] = g1

                def b_back(ctx):
                    g, g1 = ctx["g"], ctx["g1"]
                    f2_ps = psF.tile([128, 4, C], F32, tag="f2")
                    for j in range(4):
                        for ch in range(3):
                            nc.tensor.matmul(
                                f2_ps[:, j, :],
                                g1[:, ch, j * 128:(j + 1) * 128],
                                w2[:, ch, :],
                                start=(ch == 0), stop=(ch == 2),
                            )
                    out_t = sbb.tile([128, 4, C], F32, tag="outt")
                    nc.vector.scalar_tensor_tensor(
                        out=out_t[:], in0=f2_ps[:], scalar=1.0,
                        in1=x2_all[:, g], op0=ALU.mult, op1=ALU.add)
                    for wp in range(4):
                        eng = (nc.sync, nc.sync, nc.gpsimd, nc.gpsimd)[wp]
                        eng.dma_start(out=_pair_dram_ap(out_h, g, wp),
                                      in_=out_t[:, wp, :])

                bq = []
                for g in range(NGROUPS):
                    bq.append(b_front(g))
                    if len(bq) >= 2:
                        b_mid(bq[-2])
                    if len(bq) >= 3:
                        b_back(bq.pop(0))
                b_mid(bq[-1])
                b_back(bq.pop(0))
                b_back(bq.pop(0))

    nc.compile()
    return nc


def _get_program():
    if "nc" not in _CACHE:
        _CACHE["nc"] = _build_program()
    return _CACHE["nc"]


def _prep_consts(norm1_g, norm1_b, qkv_w, qkv_b, proj_w, proj_b,
                 rel_bias_table, norm2_g, norm2_b, fc1_w, fc1_b, fc2_w, fc2_b):
    # Fold LN1 affine into qkv weights; fold attention scale into the q part.
    wqkv = qkv_w * norm1_g[:, None]
    bqkv = norm1_b @ qkv_w + qkv_b            # (288,)
    wqkv = wqkv.copy()
    wqkv[:, 0:C] *= SCALE
    bqkv = bqkv.copy()
    bqkv[0:C] *= SCALE
    assert np.allclose(bqkv, 0.0), "nonzero folded qkv bias not supported"
    assert np.allclose(proj_b, 0.0) and np.allclose(fc1_b, 0.0) and np.allclose(fc2_b, 0.0), \
        "nonzero proj/fc biases not supported"

    wqk = wqkv[:, 0:2 * C]
    vW = wqkv[:, 2 * C:3 * C]
    wv = np.zeros((C + 1, 99), np.float32)
    for hh in range(HEADS):
        wv[0:C, hh * 33:hh * 33 + HD] = vW[:, hh * HD:(hh + 1) * HD]
        wv[C, hh * 33 + HD] = 1.0

    w1 = fc1_w * norm2_g[:, None]
    b1 = norm2_b @ fc1_w + fc1_b
    assert np.allclose(b1, 0.0), "nonzero folded fc1 bias not supported"

    # additive bias^T blocks: logbT[q', h, k] = bias[q, k, h] when q' and k
    # are in the same window of the pair, NEG (mask) otherwise. Token order
    # on partitions is r-major: p = r*16 + par*8 + c.
    bias = rel_bias_table[REL_IDX]            # (64, 64, HEADS) [q, k, h]
    pidx = np.arange(128)
    tok = (pidx // 16) * 8 + (pidx % 8)       # token index within window
    par = (pidx // 8) % 2                     # which window of the pair
    same = par[:, None] == par[None, :]       # [q', k]
    logbT = np.where(same[:, None, :],
                     bias[tok[:, None], tok[None, :]].transpose(0, 2, 1),
                     NEG).astype(np.float32)  # [q', h, k]

    w2 = fc2_w.reshape(3, 128, C)

    return {
        "wqk": wqk,
        "wv": wv,
        "wproj": proj_w,
        "w1": w1,
        "w2": w2,
        "logbT": logbT,
    }


def _to_bf16(a):
    import ml_dtypes
    return np.asarray(a, dtype=np.float32).astype(ml_dtypes.bfloat16)


LAST_RESULTS = None


def kernel(**inputs):
    global LAST_RESULTS
    x = np.asarray(inputs["x"], np.float32)
    consts = _prep_consts(
        np.asarray(inputs["norm1_g"], np.float32), np.asarray(inputs["norm1_b"], np.float32),
        np.asarray(inputs["qkv_w"], np.float32), np.asarray(inputs["qkv_b"], np.float32),
        np.asarray(inputs["proj_w"], np.float32), np.asarray(inputs["proj_b"], np.float32),
        np.asarray(inputs["rel_bias_table"], np.float32),
        np.asarray(inputs["norm2_g"], np.float32), np.asarray(inputs["norm2_b"], np.float32),
        np.asarray(inputs["fc1_w"], np.float32), np.asarray(inputs["fc1_b"], np.float32),
        np.asarray(inputs["fc2_w"], np.float32), np.asarray(inputs["fc2_b"], np.float32),
    )

    shared = {
        "wqk": _to_bf16(consts["wqk"]),
        "wv": _to_bf16(consts["wv"]),
        "wproj": _to_bf16(consts["wproj"]),
        "w1": _to_bf16(consts["w1"]),
        "w2": _to_bf16(consts["w2"]),
        "logbT": _to_bf16(consts["logbT"]),
        "ident": _to_bf16(np.eye(128, dtype=np.float32)),
        "identrep": _to_bf16(np.tile(np.eye(128, dtype=np.float32), (1, 4))),
    }

    xr = x.reshape(B * H, W, C)
    in_maps = []
    for c in range(NCORES):
        m = dict(shared)
        m["x"] = np.ascontiguousarray(xr[c * ROWS:(c + 1) * ROWS])
        in_maps.append(m)

    nc = _get_program()
    import os
    trace = bool(os.environ.get("KERNEL_TRACE"))
    res = bass_utils.run_bass_kernel_spmd(nc, in_maps, core_ids=list(range(NCORES)),
                                          trace=trace)
    LAST_RESULTS = res
    out = np.concatenate([r["out"] for r in res.results], axis=0)
    return out.reshape(B, H, W, C)


if __name__ == "__main__":
    print("building program...")
    _get_program()
    print("program built ok")


# revision 39
# speedup vs baseline: 1.2262x; 1.2262x over previous
"""Trainium2 Bass kernel for a Swin-style local-window ViT block.

Problem (hardcoded): x (4, 256, 256, 96) fp32, 8x8 windows, 3 heads (hd=32),
LN -> window attention (+rel-pos bias) -> proj -> residual -> LN -> MLP(4x,
gelu) -> residual.

Sharding: data-parallel. (B*H)=1024 image rows are split into 8 slabs of 128
rows; each slab holds 512 complete 8x8 windows, so the 8 cores are fully
independent (weights replicated).

v2 design notes (vs the first working version):
  - One DMA per window-pair ([[768,2],[24576,8],[1,768]]) -> 4 loads + 4
    stores per 512-token group instead of 16, dispatched on SP (loads) and
    SP/DVE/Pool (stores).
  - x2 (post-attention residual) stays resident in SBUF (96 KB/partition),
    removing the DRAM round trip entirely.
  - Scores matmuls merge the two windows of a pair (12 instead of 24 per
    group); the rel-pos bias is ADDED in PSUM by a second accumulating
    matmul (stationary = bias^T, moving = identity) with -30 as the
    cross-window mask, replacing the big DVE exp-bias multiply.
  - attn@V merges pairs too (12 matmuls) and the softmax denominator is
    produced by an appended ones column (h_t carries a constant-1 channel
    and the V weight block an extra unit column per head), so no separate
    row-sum matmuls.
  - bn_aggr is replaced by 5 tiny strided DVE ops that merge bn_stats'
    two half-triples exactly.
  - PSUM->SBUF copies ride the scalar engine so PE consumers see
    single-engine dependencies.
"""

import sys

sys.path.insert(0, "/opt/trn_rl_repo")

import numpy as np

import concourse.bass as bass
import concourse.bacc as bacc
import concourse.tile as tile
from concourse import mybir
from concourse import bass_utils

F32 = mybir.dt.float32
BF16 = mybir.dt.bfloat16
AF = mybir.ActivationFunctionType
ALU = mybir.AluOpType

B, H, W, C = 4, 256, 256, 96
WIN = 8
HEADS = 3
HD = 32
SCALE = HD ** -0.5
HID = 4 * C

NCORES = 8
ROWS = (B * H) // NCORES          # 128 image rows per core
NGROUPS = 64                      # groups of 512 tokens (8 windows) per core
SUPER = 8                         # groups per rstd batch
EPS = 1e-5
NEG = -30.0                       # additive mask for cross-window keys


def _rel_pos_index():
    coords = np.stack(np.meshgrid(np.arange(WIN), np.arange(WIN), indexing="ij")).reshape(2, -1)
    rel = coords[:, :, None] - coords[:, None, :]
    rel = rel.transpose(1, 2, 0).astype(np.int64)
    rel[:, :, 0] += WIN - 1
    rel[:, :, 1] += WIN - 1
    rel[:, :, 0] *= 2 * WIN - 1
    return rel.sum(-1)  # (64, 64)


REL_IDX = _rel_pos_index()

_CACHE = {}


def _pair_dram_ap(handle, g, wp):
    """DRAM AP for window-pair wp of group g: [8(r), 1536].

    Element order matches SBUF [128, 96] token-major with partition
    p = r*16 + par*8 + c (r-major token order), then 96 channels.
    The balanced refinement is [r 8][p-chunk 16][ch 96] = 3 dims.
    """
    wr, gc = divmod(g, 4)
    r0 = wr * WIN
    c0 = gc * 64 + wp * 16
    off = (r0 * W + c0) * C
    return bass.AP(
        tensor=handle,
        offset=off,
        ap=[[W * C, WIN], [1, 16 * C]],
    )


def _tok_idx(p):
    """Token index within its window for partition p (r-major order)."""
    return (p // 16) * 8 + (p % 8)


def _tok_par(p):
    """Which window of the pair (0/1) partition p belongs to."""
    return (p // 8) % 2


def _merge_halves(nc, st_tensor, st_off, st_pap, n, var_out, msum_out, scr):
    """Exact merge of bn_stats' two half-triples for n windows at once.

    Stats at st_off: [n, 6] per partition = (cnt, mean, M2) x 2 halves per
    window. Produces var_out [128, n] = 96*variance (M21 + M22 +
    24*(m1-m2)^2) and msum_out [128, n] = m1 + m2 (mean = msum/2).
    """
    def f(k):
        return bass.AP(tensor=st_tensor, offset=st_off + k,
                       ap=[st_pap, [6, n], [1, 1]])
    m1, M21, m2, M22 = f(1), f(2), f(4), f(5)
    dm = scr[:, 0:n]
    d2 = scr[:, n:2 * n]
    t = scr[:, 2 * n:3 * n]
    nc.vector.tensor_tensor(out=dm, in0=m1, in1=m2, op=ALU.subtract)
    nc.vector.tensor_tensor(out=msum_out, in0=m1, in1=m2, op=ALU.add)
    nc.vector.tensor_tensor(out=d2, in0=dm, in1=dm, op=ALU.mult)
    nc.vector.tensor_tensor(out=t, in0=M21, in1=M22, op=ALU.add)
    nc.vector.scalar_tensor_tensor(out=var_out, in0=d2, scalar=24.0, in1=t,
                                   op0=ALU.mult, op1=ALU.add)


def _build_program():
    nc = bacc.Bacc("TRN2", target_bir_lowering=False, debug=False)

    x_h = nc.dram_tensor("x", [ROWS, W, C], F32, kind="ExternalInput")
    out_h = nc.dram_tensor("out", [ROWS, W, C], F32, kind="ExternalOutput")

    wqk_h = nc.dram_tensor("wqk", [C, 2 * C], BF16, kind="ExternalInput")
    wv_h = nc.dram_tensor("wv", [C + 1, 99], BF16, kind="ExternalInput")
    wp_h = nc.dram_tensor("wproj", [C, C], BF16, kind="ExternalInput")
    w1_h = nc.dram_tensor("w1", [C, HID], BF16, kind="ExternalInput")
    w2_h = nc.dram_tensor("w2", [3, 128, C], BF16, kind="ExternalInput")
    logb_h = nc.dram_tensor("logbT", [128, 3, 128], BF16, kind="ExternalInput")
    ident_h = nc.dram_tensor("ident", [128, 128], BF16, kind="ExternalInput")
    identrep_h = nc.dram_tensor("identrep", [128, 512], BF16, kind="ExternalInput")

    with tile.TileContext(nc) as tc:
        with tc.tile_pool(name="const", bufs=1) as cpool:
            wqk = cpool.tile([C, 2 * C], BF16)
            nc.sync.dma_start(out=wqk, in_=wqk_h.ap())
            wv = cpool.tile([C + 1, 99], BF16)
            nc.sync.dma_start(out=wv, in_=wv_h.ap())
            wproj = cpool.tile([C, C], BF16)
            nc.sync.dma_start(out=wproj, in_=wp_h.ap())
            w1 = cpool.tile([C, HID], BF16)
            nc.sync.dma_start(out=w1, in_=w1_h.ap())
            w2 = cpool.tile([128, 3, C], BF16)
            nc.sync.dma_start(out=w2, in_=w2_h.ap().rearrange("c p f -> p c f"))
            logb = cpool.tile([128, 3, 128], BF16)
            nc.sync.dma_start(out=logb, in_=logb_h.ap())
            ident = cpool.tile([128, 128], BF16)
            nc.sync.dma_start(out=ident, in_=ident_h.ap())
            identrep = cpool.tile([128, 512], BF16)
            nc.sync.dma_start(out=identrep, in_=identrep_h.ap())
            epsb = cpool.tile([128, 1], F32)
            nc.vector.memset(epsb, EPS)

            x2_all = cpool.tile([128, NGROUPS, 4, C], BF16)
            st2_all = cpool.tile([128, NGROUPS, 4, 6], F32)
            var2_all = cpool.tile([128, NGROUPS * 4], F32)
            rstd2_all = cpool.tile([128, NGROUPS * 4], F32)
            nmr2_all = cpool.tile([128, NGROUPS * 4], F32)
            scr2_all = cpool.tile([128, NGROUPS * 12], F32)

            # Warm-up: make PE/ACT observe each const-load DMA semaphore via a
            # tiny op, so real instructions never need two sync waits (this
            # walrus build supports at most one per instruction).
            with tc.tile_pool(name="warm", bufs=1, space="PSUM") as wps:
                wp_t = wps.tile([1, 8], F32)
                def _tiny(t):
                    base = t[:]
                    return bass.AP(tensor=base.tensor, offset=base.offset,
                                   ap=[[base.ap[0][0], 1], [1, 1]])
                for ci, cst in enumerate((wqk, wv, wproj, w1, w2, logb, ident,
                                          identrep)):
                    nc.tensor.matmul(wp_t[0:1, ci:ci + 1], _tiny(cst),
                                     _tiny(cst), start=True, stop=True)
                wd = cpool.tile([1, 1], BF16)
                nc.scalar.activation(out=wd, in_=_tiny(logb), func=AF.Copy)

            # ---------------- Phase A: attention ----------------
            with (
                tc.tile_pool(name="xin", bufs=SUPER + 5) as xpool,
                tc.tile_pool(name="stat", bufs=SUPER + 2) as stpool,
                tc.tile_pool(name="vs", bufs=2) as vspool,
                tc.tile_pool(name="sba", bufs=2) as sba,
                tc.tile_pool(name="psW", bufs=5, space="PSUM") as psW,
                tc.tile_pool(name="psS", bufs=1, space="PSUM") as psS,
            ):
                def do_front(g, x_t, rstd1, nmr_all, gi):
                    # LN1 apply: h = x*rstd + nmr (2 windows on ACT, 2 on DVE)
                    h_t = sba.tile([128, 4, C + 1], BF16, tag="h")
                    for j in range(4):
                        if j % 2 == 0:
                            nc.scalar.activation(
                                out=h_t[:, j, 0:C], in_=x_t[:, j, :],
                                func=AF.Identity,
                                scale=rstd1[:, gi * 4 + j:gi * 4 + j + 1],
                                bias=nmr_all[:, gi * 4 + j:gi * 4 + j + 1])
                        else:
                            nc.vector.tensor_scalar(
                                out=h_t[:, j, 0:C], in0=x_t[:, j, :],
                                scalar1=rstd1[:, gi * 4 + j:gi * 4 + j + 1],
                                scalar2=nmr_all[:, gi * 4 + j:gi * 4 + j + 1],
                                op0=ALU.mult, op1=ALU.add)
                    # column C stays 1.0: the two ring buffers for tag "h"
                    # were pre-initialized below and nothing else writes it

                    hT_ps = psW.tile([C + 1, 512], BF16, tag="w")
                    for j in range(4):
                        nc.tensor.transpose(hT_ps[:, j * 128:(j + 1) * 128],
                                            h_t[:, j, :], ident[:])
                    hT = sba.tile([C + 1, 512], BF16, tag="hT")
                    nc.vector.tensor_copy(out=hT, in_=hT_ps[:])

                    qT_ps = psW.tile([C, 512], F32, tag="w")
                    nc.tensor.matmul(qT_ps[:], wqk[:, 0:C], hT[0:C, :],
                                     start=True, stop=True)
                    kT_ps = psW.tile([C, 512], F32, tag="w")
                    nc.tensor.matmul(kT_ps[:], wqk[:, C:2 * C], hT[0:C, :],
                                     start=True, stop=True)
                    v_ps = psW.tile([128, 4, 99], F32, tag="w")
                    for j in range(4):
                        nc.tensor.matmul(v_ps[:, j, :],
                                         hT[:, j * 128:(j + 1) * 128],
                                         wv[:], start=True, stop=True)
                    qT = sba.tile([C, 512], BF16, tag="qT")
                    nc.scalar.activation(out=qT, in_=qT_ps[:], func=AF.Copy)
                    kT = sba.tile([C, 512], BF16, tag="kT")
                    nc.scalar.activation(out=kT, in_=kT_ps[:], func=AF.Copy)
                    v_t = sba.tile([128, 4, 99], BF16, tag="v")
                    nc.vector.tensor_copy(out=v_t, in_=v_ps[:])
                    return dict(g=g, x_t=x_t, qT=qT, kT=kT, v_t=v_t)

                def do_mid(ctx):
                    qT, kT = ctx["qT"], ctx["kT"]
                    sc_ps = psS.tile([128, 3, 512], F32, tag="sc")
                    for wp in range(4):
                        for hh in range(HEADS):
                            nc.tensor.matmul(
                                sc_ps[:, hh, wp * 128:(wp + 1) * 128],
                                kT[hh * HD:(hh + 1) * HD, wp * 128:(wp + 1) * 128],
                                qT[hh * HD:(hh + 1) * HD, wp * 128:(wp + 1) * 128],
                                start=True, stop=False,
                                tile_position=(hh * HD, 0),
                            )
                    for hh in range(HEADS):
                        nc.tensor.matmul(
                            sc_ps[:, hh, :], logb[:, hh, :], identrep[:],
                            start=False, stop=True,
                        )
                    E_t = sba.tile([128, 3, 512], BF16, tag="E")
                    nc.scalar.activation(out=E_t, in_=sc_ps[:], func=AF.Exp)
                    ctx["E_t"] = E_t

                def do_back1(ctx):
                    E_t, v_t = ctx["E_t"], ctx["v_t"]
                    o_ps = psW.tile([128, 4, 99], F32, tag="w")
                    for wp in range(4):
                        for hh in range(HEADS):
                            nc.tensor.matmul(
                                o_ps[:, wp, hh * 33:(hh + 1) * 33],
                                E_t[:, hh, wp * 128:(wp + 1) * 128],
                                v_t[:, wp, hh * 33:(hh + 1) * 33],
                                start=True, stop=True,
                            )
                    rs = stpool.tile([128, 12], F32, tag="rs")
                    s_ap = bass.AP(tensor=o_ps.tensor,
                                   offset=o_ps[:].offset + 32,
                                   ap=[o_ps[:].ap[0], [99, 4], [33, 3], [1, 1]])
                    nc.vector.reciprocal(out=rs, in_=s_ap)
                    o_t = sba.tile([128, 4, C], BF16, tag="o")
                    o_src = bass.AP(tensor=o_ps.tensor, offset=o_ps[:].offset,
                                    ap=[o_ps[:].ap[0], [99, 4], [33, 3], [1, HD]])
                    rs_b = bass.AP(tensor=rs.tensor, offset=rs[:].offset,
                                   ap=[rs[:].ap[0], [3, 4], [1, 3], [0, HD]])
                    o_dst = o_t[:].rearrange("p a (h d) -> p a h d", h=HEADS)
                    nc.vector.tensor_tensor(out=o_dst, in0=o_src, in1=rs_b,
                                            op=ALU.mult)
                    ctx["o_t"] = o_t

                def do_back2(ctx):
                    g, o_t, x_t = ctx["g"], ctx["o_t"], ctx["x_t"]
                    oT_ps = psW.tile([C, 512], BF16, tag="w")
                    for j in range(4):
                        nc.tensor.transpose(oT_ps[:, j * 128:(j + 1) * 128],
                                            o_t[:, j, :], ident[:])
                    oT = sba.tile([C, 512], BF16, tag="oT")
                    nc.vector.tensor_copy(out=oT, in_=oT_ps[:])

                    att_ps = psW.tile([128, 4, C], F32, tag="w")
                    for j in range(4):
                        nc.tensor.matmul(att_ps[:, j, :],
                                         oT[:, j * 128:(j + 1) * 128],
                                         wproj[:], start=True, stop=True)
                    nc.vector.scalar_tensor_tensor(
                        out=x2_all[:, g], in0=att_ps[:], scalar=1.0,
                        in1=x_t[:], op0=ALU.mult, op1=ALU.add)
                    for j in range(4):
                        nc.vector.bn_stats(out=st2_all[:, g, j, :],
                                           in_=x2_all[:, g, j, :])

                # pre-initialize the constant-1 column in the two "h" ring
                # buffers (LN applies never touch column C, transposes read it)
                for _ in range(2):
                    h_init = sba.tile([128, 4, C + 1], BF16, tag="h")
                    nc.vector.memset(h_init[:, :, C:C + 1], 1.0)

                inflight = []
                for sb in range(NGROUPS // SUPER):
                    x_ts = []
                    stS = stpool.tile([128, SUPER, 4, 6], F32, tag="st")
                    scr = stpool.tile([128, SUPER * 12], F32, tag="scr")
                    var1 = vspool.tile([128, SUPER * 4], F32, tag="var1")
                    msum = vspool.tile([128, SUPER * 4], F32, tag="msum")
                    for gi in range(SUPER):
                        g = sb * SUPER + gi
                        x_t = xpool.tile([128, 4, C], F32, tag="x")
                        for wp in range(4):
                            nc.sync.dma_start(out=x_t[:, wp, :],
                                              in_=_pair_dram_ap(x_h, g, wp))
                        for j in range(4):
                            nc.vector.bn_stats(out=stS[:, gi, j, :],
                                               in_=x_t[:, j, :])
                        x_ts.append(x_t)
                    # one batched merge for all SUPER*4 windows
                    _merge_halves(nc, stS.tensor, stS[:].offset, stS[:].ap[0],
                                  SUPER * 4, var1[:], msum[:], scr)
                    # batched rstd = exp(-0.5 * ln(var96/96 + eps))
                    lnv = vspool.tile([128, SUPER * 4], F32, tag="lnv")
                    nc.scalar.activation(out=lnv, in_=var1[:], func=AF.Ln,
                                         bias=epsb[:], scale=1.0 / 96.0)
                    rstd1 = vspool.tile([128, SUPER * 4], F32, tag="rstd1")
                    nc.scalar.activation(out=rstd1, in_=lnv[:], func=AF.Exp,
                                         scale=-0.5)
                    # nmr = -mean*rstd = -0.5*msum*rstd (batched)
                    nmr_all = vspool.tile([128, SUPER * 4], F32, tag="nmr")
                    nc.vector.scalar_tensor_tensor(
                        out=nmr_all, in0=msum[:], scalar=-0.5,
                        in1=rstd1[:], op0=ALU.mult, op1=ALU.mult)

                    for gi in range(SUPER):
                        g = sb * SUPER + gi
                        inflight.append(do_front(g, x_ts[gi], rstd1,
                                                 nmr_all, gi))
                        if len(inflight) >= 2:
                            do_mid(inflight[-2])
                        if len(inflight) >= 3:
                            do_back1(inflight[-3])
                        if len(inflight) >= 4:
                            do_back2(inflight.pop(0))
                # pipeline flush
                do_mid(inflight[-1])
                do_back1(inflight[-2])
                do_back2(inflight.pop(0))
                do_back1(inflight[-1])
                do_back2(inflight.pop(0))
                do_back2(inflight.pop(0))

                # batched LN2 stats merge + rstd for ALL groups at once
                # (still on the exp/ln table)
                _merge_halves(nc, st2_all.tensor, st2_all[:].offset,
                              st2_all[:].ap[0], NGROUPS * 4,
                              var2_all[:], nmr2_all[:], scr2_all)
                lnv2 = cpool.tile([128, NGROUPS * 4], F32)
                nc.scalar.activation(out=lnv2, in_=var2_all[:], func=AF.Ln,
                                     bias=epsb[:], scale=1.0 / 96.0)
                nc.scalar.activation(out=rstd2_all[:], in_=lnv2[:], func=AF.Exp,
                                     scale=-0.5)
                # nmr2 = -mean*rstd = -0.5*meansum*rstd
                nc.vector.scalar_tensor_tensor(
                    out=nmr2_all[:], in0=nmr2_all[:], scalar=-0.5,
                    in1=rstd2_all[:], op0=ALU.mult, op1=ALU.mult)

            # ---------------- Phase B: MLP ----------------
            with (
                tc.tile_pool(name="sbb", bufs=2) as sbb,
                tc.tile_pool(name="psWB", bufs=2, space="PSUM") as psWB,
                tc.tile_pool(name="psG", bufs=2, space="PSUM") as psG,
            ):
                def b_front(g):
                    h2 = sbb.tile([128, 4, C], BF16, tag="h2")
                    for j in range(4):
                        nc.vector.tensor_scalar(
                            out=h2[:, j, :], in0=x2_all[:, g, j, :],
                            scalar1=rstd2_all[:, g * 4 + j:g * 4 + j + 1],
                            scalar2=nmr2_all[:, g * 4 + j:g * 4 + j + 1],
                            op0=ALU.mult, op1=ALU.add)
                    h2T_ps = psWB.tile([C, 512], BF16, tag="wb")
                    for j in range(4):
                        nc.tensor.transpose(h2T_ps[:, j * 128:(j + 1) * 128],
                                            h2[:, j, :], ident[:])
                    h2T = sbb.tile([C, 512], BF16, tag="h2T")
                    nc.vector.tensor_copy(out=h2T, in_=h2T_ps[:])
                    return dict(g=g, h2T=h2T)

                def b_mid(ctx):
                    g1_ps = psG.tile([128, 3, 512], F32, tag="g1")
                    for ch in range(3):
                        nc.tensor.matmul(g1_ps[:, ch, :],
                                         w1[:, ch * 128:(ch + 1) * 128],
                                         ctx["h2T"][:], start=True, stop=True)
                    g1 = sbb.tile([128, 3, 512], BF16, tag="g1s")
                    nc.scalar.activation(out=g1, in_=g1_ps[:], func=AF.Gelu)
                    ctx["g1"] = g1

                def b_back(ctx):
                    g, g1 = ctx["g"], ctx["g1"]
                    f2_ps = psWB.tile([128, 4, C], F32, tag="wb")
                    for j in range(4):
                        for ch in range(3):
                            nc.tensor.matmul(
                                f2_ps[:, j, :],
                                g1[:, ch, j * 128:(j + 1) * 128],
                                w2[:, ch, :],
                                start=(ch == 0), stop=(ch == 2),
                            )
                    out_t = sbb.tile([128, 4, C], F32, tag="outt")
                    nc.vector.scalar_tensor_tensor(
                        out=out_t[:], in0=f2_ps[:], scalar=1.0,
                        in1=x2_all[:, g], op0=ALU.mult, op1=ALU.add)
                    for wp in range(4):
                        eng = (nc.sync, nc.sync, nc.gpsimd, nc.gpsimd)[wp]
                        eng.dma_start(out=_pair_dram_ap(out_h, g, wp),
                                      in_=out_t[:, wp, :])

                bq = []
                for g in range(NGROUPS):
                    bq.append(b_front(g))
                    if len(bq) >= 2:
                        b_mid(bq[-2])
                    if len(bq) >= 3:
                        b_back(bq.pop(0))
                b_mid(bq[-1])
                b_back(bq.pop(0))
                b_back(bq.pop(0))

    nc.compile()
    return nc


def _get_program():
    if "nc" not in _CACHE:
        _CACHE["nc"] = _build_program()
    return _CACHE["nc"]


def _prep_consts(norm1_g, norm1_b, qkv_w, qkv_b, proj_w, proj_b,
                 rel_bias_table, norm2_g, norm2_b, fc1_w, fc1_b, fc2_w, fc2_b):
    # Fold LN1 affine into qkv weights; fold attention scale into the q part.
    wqkv = qkv_w * norm1_g[:, None]
    bqkv = norm1_b @ qkv_w + qkv_b            # (288,)
    wqkv = wqkv.copy()
    wqkv[:, 0:C] *= SCALE
    bqkv = bqkv.copy()
    bqkv[0:C] *= SCALE
    assert np.allclose(bqkv, 0.0), "nonzero folded qkv bias not supported"
    assert np.allclose(proj_b, 0.0) and np.allclose(fc1_b, 0.0) and np.allclose(fc2_b, 0.0), \
        "nonzero proj/fc biases not supported"

    wqk = wqkv[:, 0:2 * C]
    vW = wqkv[:, 2 * C:3 * C]
    wv = np.zeros((C + 1, 99), np.float32)
    for hh in range(HEADS):
        wv[0:C, hh * 33:hh * 33 + HD] = vW[:, hh * HD:(hh + 1) * HD]
        wv[C, hh * 33 + HD] = 1.0

    w1 = fc1_w * norm2_g[:, None]
    b1 = norm2_b @ fc1_w + fc1_b
    assert np.allclose(b1, 0.0), "nonzero folded fc1 bias not supported"

    # additive bias^T blocks: logbT[q', h, k] = bias[q, k, h] when q' and k
    # are in the same window of the pair, NEG (mask) otherwise. Token order
    # on partitions is r-major: p = r*16 + par*8 + c.
    bias = rel_bias_table[REL_IDX]            # (64, 64, HEADS) [q, k, h]
    pidx = np.arange(128)
    tok = (pidx // 16) * 8 + (pidx % 8)       # token index within window
    par = (pidx // 8) % 2                     # which window of the pair
    same = par[:, None] == par[None, :]       # [q', k]
    logbT = np.where(same[:, None, :],
                     bias[tok[:, None], tok[None, :]].transpose(0, 2, 1),
                     NEG).astype(np.float32)  # [q', h, k]

    w2 = fc2_w.reshape(3, 128, C)

    return {
        "wqk": wqk,
        "wv": wv,
        "wproj": proj_w,
        "w1": w1,
        "w2": w2,
        "logbT": logbT,
    }


def _to_bf16(a):
    import ml_dtypes
    return np.asarray(a, dtype=np.float32).astype(ml_dtypes.bfloat16)


LAST_RESULTS = None


def kernel(**inputs):
    global LAST_RESULTS
    x = np.asarray(inputs["x"], np.float32)
    consts = _prep_consts(
        np.asarray(inputs["norm1_g"], np.float32), np.asarray(inputs["norm1_b"], np.float32),
        np.asarray(inputs["qkv_w"], np.float32), np.asarray(inputs["qkv_b"], np.float32),
        np.asarray(inputs["proj_w"], np.float32), np.asarray(inputs["proj_b"], np.float32),
        np.asarray(inputs["rel_bias_table"], np.float32),
        np.asarray(inputs["norm2_g"], np.float32), np.asarray(inputs["norm2_b"], np.float32),
        np.asarray(inputs["fc1_w"], np.float32), np.asarray(inputs["fc1_b"], np.float32),
        np.asarray(inputs["fc2_w"], np.float32), np.asarray(inputs["fc2_b"], np.float32),
    )

    shared = {
        "wqk": _to_bf16(consts["wqk"]),
        "wv": _to_bf16(consts["wv"]),
        "wproj": _to_bf16(consts["wproj"]),
        "w1": _to_bf16(consts["w1"]),
        "w2": _to_bf16(consts["w2"]),
        "logbT": _to_bf16(consts["logbT"]),
        "ident": _to_bf16(np.eye(128, dtype=np.float32)),
        "identrep": _to_bf16(np.tile(np.eye(128, dtype=np.float32), (1, 4))),
    }

    xr = x.reshape(B * H, W, C)
    in_maps = []
    for c in range(NCORES):
        m = dict(shared)
        m["x"] = np.ascontiguousarray(xr[c * ROWS:(c + 1) * ROWS])
        in_maps.append(m)

    nc = _get_program()
    import os
    trace = bool(os.environ.get("KERNEL_TRACE"))
    res = bass_utils.run_bass_kernel_spmd(nc, in_maps, core_ids=list(range(NCORES)),
                                          trace=trace)
    LAST_RESULTS = res
    out = np.concatenate([r["out"] for r in res.results], axis=0)
    return out.reshape(B, H, W, C)


if __name__ == "__main__":
    print("building program...")
    _get_program()
    print("program built ok")


# revision 42
# speedup vs baseline: 1.2427x; 1.0134x over previous
"""Trainium2 Bass kernel for a Swin-style local-window ViT block.

Problem (hardcoded): x (4, 256, 256, 96) fp32, 8x8 windows, 3 heads (hd=32),
LN -> window attention (+rel-pos bias) -> proj -> residual -> LN -> MLP(4x,
gelu) -> residual.

Sharding: data-parallel. (B*H)=1024 image rows are split into 8 slabs of 128
rows; each slab holds 512 complete 8x8 windows, so the 8 cores are fully
independent (weights replicated).

v2 design notes (vs the first working version):
  - One DMA per window-pair ([[768,2],[24576,8],[1,768]]) -> 4 loads + 4
    stores per 512-token group instead of 16, dispatched on SP (loads) and
    SP/DVE/Pool (stores).
  - x2 (post-attention residual) stays resident in SBUF (96 KB/partition),
    removing the DRAM round trip entirely.
  - Scores matmuls merge the two windows of a pair (12 instead of 24 per
    group); the rel-pos bias is ADDED in PSUM by a second accumulating
    matmul (stationary = bias^T, moving = identity) with -30 as the
    cross-window mask, replacing the big DVE exp-bias multiply.
  - attn@V merges pairs too (12 matmuls) and the softmax denominator is
    produced by an appended ones column (h_t carries a constant-1 channel
    and the V weight block an extra unit column per head), so no separate
    row-sum matmuls.
  - bn_aggr is replaced by 5 tiny strided DVE ops that merge bn_stats'
    two half-triples exactly.
  - PSUM->SBUF copies ride the scalar engine so PE consumers see
    single-engine dependencies.
"""

import sys

sys.path.insert(0, "/opt/trn_rl_repo")

import numpy as np

import concourse.bass as bass
import concourse.bacc as bacc
import concourse.tile as tile
from concourse import mybir
from concourse import bass_utils

F32 = mybir.dt.float32
BF16 = mybir.dt.bfloat16
AF = mybir.ActivationFunctionType
ALU = mybir.AluOpType

B, H, W, C = 4, 256, 256, 96
WIN = 8
HEADS = 3
HD = 32
SCALE = HD ** -0.5
HID = 4 * C

NCORES = 8
ROWS = (B * H) // NCORES          # 128 image rows per core
NGROUPS = 64                      # groups of 512 tokens (8 windows) per core
SUPER = 8                         # groups per rstd batch
EPS = 1e-5
NEG = -30.0                       # additive mask for cross-window keys


def _rel_pos_index():
    coords = np.stack(np.meshgrid(np.arange(WIN), np.arange(WIN), indexing="ij")).reshape(2, -1)
    rel = coords[:, :, None] - coords[:, None, :]
    rel = rel.transpose(1, 2, 0).astype(np.int64)
    rel[:, :, 0] += WIN - 1
    rel[:, :, 1] += WIN - 1
    rel[:, :, 0] *= 2 * WIN - 1
    return rel.sum(-1)  # (64, 64)


REL_IDX = _rel_pos_index()

_CACHE = {}


def _pair_dram_ap(handle, g, wp):
    """DRAM AP for window-pair wp of group g: [8(r), 1536].

    Element order matches SBUF [128, 96] token-major with partition
    p = r*16 + par*8 + c (r-major token order), then 96 channels.
    The balanced refinement is [r 8][p-chunk 16][ch 96] = 3 dims.
    """
    wr, gc = divmod(g, 4)
    r0 = wr * WIN
    c0 = gc * 64 + wp * 16
    off = (r0 * W + c0) * C
    return bass.AP(
        tensor=handle,
        offset=off,
        ap=[[W * C, WIN], [1, 16 * C]],
    )


def _tok_idx(p):
    """Token index within its window for partition p (r-major order)."""
    return (p // 16) * 8 + (p % 8)


def _tok_par(p):
    """Which window of the pair (0/1) partition p belongs to."""
    return (p // 8) % 2


def _merge_halves(nc, st_tensor, st_off, st_pap, n, var_out, msum_out, scr):
    """Exact merge of bn_stats' two half-triples for n windows at once.

    Stats at st_off: [n, 6] per partition = (cnt, mean, M2) x 2 halves per
    window. Produces var_out [128, n] = 96*variance (M21 + M22 +
    24*(m1-m2)^2) and msum_out [128, n] = m1 + m2 (mean = msum/2).
    """
    def f(k):
        return bass.AP(tensor=st_tensor, offset=st_off + k,
                       ap=[st_pap, [6, n], [1, 1]])
    m1, M21, m2, M22 = f(1), f(2), f(4), f(5)
    dm = scr[:, 0:n]
    d2 = scr[:, n:2 * n]
    t = scr[:, 2 * n:3 * n]
    nc.vector.tensor_tensor(out=dm, in0=m1, in1=m2, op=ALU.subtract)
    nc.vector.tensor_tensor(out=msum_out, in0=m1, in1=m2, op=ALU.add)
    nc.vector.tensor_tensor(out=d2, in0=dm, in1=dm, op=ALU.mult)
    nc.vector.tensor_tensor(out=t, in0=M21, in1=M22, op=ALU.add)
    nc.vector.scalar_tensor_tensor(out=var_out, in0=d2, scalar=24.0, in1=t,
                                   op0=ALU.mult, op1=ALU.add)


def _build_program():
    nc = bacc.Bacc("TRN2", target_bir_lowering=False, debug=False)

    x_h = nc.dram_tensor("x", [ROWS, W, C], F32, kind="ExternalInput")
    out_h = nc.dram_tensor("out", [ROWS, W, C], F32, kind="ExternalOutput")

    wqk_h = nc.dram_tensor("wqk", [C, 2 * C], BF16, kind="ExternalInput")
    wv_h = nc.dram_tensor("wv", [C + 1, 99], BF16, kind="ExternalInput")
    wp_h = nc.dram_tensor("wproj", [C, C], BF16, kind="ExternalInput")
    w1_h = nc.dram_tensor("w1", [C, HID], BF16, kind="ExternalInput")
    w2_h = nc.dram_tensor("w2", [3, 128, C], BF16, kind="ExternalInput")
    logb_h = nc.dram_tensor("logbT", [128, 3, 128], BF16, kind="ExternalInput")
    ident_h = nc.dram_tensor("ident", [128, 128], BF16, kind="ExternalInput")
    identrep_h = nc.dram_tensor("identrep", [128, 512], BF16, kind="ExternalInput")

    with tile.TileContext(nc) as tc:
        with tc.tile_pool(name="const", bufs=1) as cpool:
            wqk = cpool.tile([C, 2 * C], BF16)
            nc.sync.dma_start(out=wqk, in_=wqk_h.ap())
            wv = cpool.tile([C + 1, 99], BF16)
            nc.sync.dma_start(out=wv, in_=wv_h.ap())
            wproj = cpool.tile([C, C], BF16)
            nc.sync.dma_start(out=wproj, in_=wp_h.ap())
            w1 = cpool.tile([C, HID], BF16)
            nc.sync.dma_start(out=w1, in_=w1_h.ap())
            w2 = cpool.tile([128, 3, C], BF16)
            nc.sync.dma_start(out=w2, in_=w2_h.ap().rearrange("c p f -> p c f"))
            logb = cpool.tile([128, 3, 128], BF16)
            nc.sync.dma_start(out=logb, in_=logb_h.ap())
            ident = cpool.tile([128, 128], BF16)
            nc.sync.dma_start(out=ident, in_=ident_h.ap())
            identrep = cpool.tile([128, 512], BF16)
            nc.sync.dma_start(out=identrep, in_=identrep_h.ap())
            epsb = cpool.tile([128, 1], F32)
            nc.vector.memset(epsb, EPS)

            x2_all = cpool.tile([128, NGROUPS, 4, C], BF16)
            st2_all = cpool.tile([128, NGROUPS, 4, 6], F32)
            var2_all = cpool.tile([128, NGROUPS * 4], F32)
            rstd2_all = cpool.tile([128, NGROUPS * 4], F32)
            nmr2_all = cpool.tile([128, NGROUPS * 4], F32)
            scr2_all = cpool.tile([128, NGROUPS * 12], F32)

            # Warm-up: make PE/ACT observe each const-load DMA semaphore via a
            # tiny op, so real instructions never need two sync waits (this
            # walrus build supports at most one per instruction).
            with tc.tile_pool(name="warm", bufs=1, space="PSUM") as wps:
                wp_t = wps.tile([1, 8], F32)
                def _tiny(t):
                    base = t[:]
                    return bass.AP(tensor=base.tensor, offset=base.offset,
                                   ap=[[base.ap[0][0], 1], [1, 1]])
                for ci, cst in enumerate((wqk, wv, wproj, w1, w2, logb, ident,
                                          identrep)):
                    nc.tensor.matmul(wp_t[0:1, ci:ci + 1], _tiny(cst),
                                     _tiny(cst), start=True, stop=True)
                wd = cpool.tile([1, 1], BF16)
                nc.scalar.activation(out=wd, in_=_tiny(logb), func=AF.Copy)

            # ---------------- Phase A: attention ----------------
            with (
                tc.tile_pool(name="xin", bufs=SUPER + 5) as xpool,
                tc.tile_pool(name="stat", bufs=SUPER + 2) as stpool,
                tc.tile_pool(name="vs", bufs=2) as vspool,
                tc.tile_pool(name="sba", bufs=2) as sba,
                tc.tile_pool(name="psW", bufs=5, space="PSUM") as psW,
                tc.tile_pool(name="psS", bufs=1, space="PSUM") as psS,
            ):
                def do_front(g, x_t, rstd1, nmr_all, gi):
                    # LN1 apply: h = x*rstd + nmr (2 windows on ACT, 2 on DVE)
                    h_t = sba.tile([128, 4, C + 1], BF16, tag="h")
                    for j in range(4):
                        if j % 2 == 0:
                            nc.scalar.activation(
                                out=h_t[:, j, 0:C], in_=x_t[:, j, :],
                                func=AF.Identity,
                                scale=rstd1[:, gi * 4 + j:gi * 4 + j + 1],
                                bias=nmr_all[:, gi * 4 + j:gi * 4 + j + 1])
                        else:
                            nc.vector.tensor_scalar(
                                out=h_t[:, j, 0:C], in0=x_t[:, j, :],
                                scalar1=rstd1[:, gi * 4 + j:gi * 4 + j + 1],
                                scalar2=nmr_all[:, gi * 4 + j:gi * 4 + j + 1],
                                op0=ALU.mult, op1=ALU.add)
                    # column C stays 1.0: the two ring buffers for tag "h"
                    # were pre-initialized below and nothing else writes it

                    hT_ps = psW.tile([C + 1, 512], BF16, tag="w")
                    for j in range(4):
                        nc.tensor.transpose(hT_ps[:, j * 128:(j + 1) * 128],
                                            h_t[:, j, :], ident[:])
                    hT = sba.tile([C + 1, 512], BF16, tag="hT")
                    nc.vector.tensor_copy(out=hT, in_=hT_ps[:])

                    qT_ps = psW.tile([C, 512], F32, tag="w")
                    nc.tensor.matmul(qT_ps[:], wqk[:, 0:C], hT[0:C, :],
                                     start=True, stop=True)
                    kT_ps = psW.tile([C, 512], F32, tag="w")
                    nc.tensor.matmul(kT_ps[:], wqk[:, C:2 * C], hT[0:C, :],
                                     start=True, stop=True)
                    v_ps = psW.tile([128, 4, 99], F32, tag="w")
                    for j in range(4):
                        nc.tensor.matmul(v_ps[:, j, :],
                                         hT[:, j * 128:(j + 1) * 128],
                                         wv[:], start=True, stop=True)
                    qT = sba.tile([C, 512], BF16, tag="qT")
                    nc.scalar.activation(out=qT, in_=qT_ps[:], func=AF.Copy)
                    kT = sba.tile([C, 512], BF16, tag="kT")
                    nc.scalar.activation(out=kT, in_=kT_ps[:], func=AF.Copy)
                    v_t = sba.tile([128, 4, 99], BF16, tag="v")
                    nc.vector.tensor_copy(out=v_t, in_=v_ps[:])
                    return dict(g=g, x_t=x_t, qT=qT, kT=kT, v_t=v_t)

                def do_mid(ctx):
                    qT, kT = ctx["qT"], ctx["kT"]
                    sc_ps = psS.tile([128, 3, 512], F32, tag="sc")
                    for wp in range(4):
                        for hh in range(HEADS):
                            nc.tensor.matmul(
                                sc_ps[:, hh, wp * 128:(wp + 1) * 128],
                                kT[hh * HD:(hh + 1) * HD, wp * 128:(wp + 1) * 128],
                                qT[hh * HD:(hh + 1) * HD, wp * 128:(wp + 1) * 128],
                                start=True, stop=False,
                                tile_position=(hh * HD, 0),
                            )
                    for hh in range(HEADS):
                        nc.tensor.matmul(
                            sc_ps[:, hh, :], logb[:, hh, :], identrep[:],
                            start=False, stop=True,
                        )
                    E_t = sba.tile([128, 3, 512], BF16, tag="E")
                    nc.scalar.activation(out=E_t, in_=sc_ps[:], func=AF.Exp)
                    ctx["E_t"] = E_t

                def do_back1(ctx):
                    E_t, v_t = ctx["E_t"], ctx["v_t"]
                    o_ps = psW.tile([128, 4, 99], F32, tag="w")
                    for wp in range(4):
                        for hh in range(HEADS):
                            nc.tensor.matmul(
                                o_ps[:, wp, hh * 33:(hh + 1) * 33],
                                E_t[:, hh, wp * 128:(wp + 1) * 128],
                                v_t[:, wp, hh * 33:(hh + 1) * 33],
                                start=True, stop=True,
                            )
                    rs = stpool.tile([128, 12], F32, tag="rs")
                    s_ap = bass.AP(tensor=o_ps.tensor,
                                   offset=o_ps[:].offset + 32,
                                   ap=[o_ps[:].ap[0], [99, 4], [33, 3], [1, 1]])
                    nc.vector.reciprocal(out=rs, in_=s_ap)
                    o_t = sba.tile([128, 4, C], BF16, tag="o")
                    o_src = bass.AP(tensor=o_ps.tensor, offset=o_ps[:].offset,
                                    ap=[o_ps[:].ap[0], [99, 4], [33, 3], [1, HD]])
                    rs_b = bass.AP(tensor=rs.tensor, offset=rs[:].offset,
                                   ap=[rs[:].ap[0], [3, 4], [1, 3], [0, HD]])
                    o_dst = o_t[:].rearrange("p a (h d) -> p a h d", h=HEADS)
                    nc.vector.tensor_tensor(out=o_dst, in0=o_src, in1=rs_b,
                                            op=ALU.mult)
                    ctx["o_t"] = o_t

                def do_back2(ctx):
                    g, o_t, x_t = ctx["g"], ctx["o_t"], ctx["x_t"]
                    oT_ps = psW.tile([C, 512], BF16, tag="w")
                    for j in range(4):
                        nc.tensor.transpose(oT_ps[:, j * 128:(j + 1) * 128],
                                            o_t[:, j, :], ident[:])
                    oT = sba.tile([C, 512], BF16, tag="oT")
                    nc.vector.tensor_copy(out=oT, in_=oT_ps[:])

                    att_ps = psW.tile([128, 4, C], F32, tag="w")
                    for j in range(4):
                        nc.tensor.matmul(att_ps[:, j, :],
                                         oT[:, j * 128:(j + 1) * 128],
                                         wproj[:], start=True, stop=True)
                    nc.vector.scalar_tensor_tensor(
                        out=x2_all[:, g], in0=att_ps[:], scalar=1.0,
                        in1=x_t[:], op0=ALU.mult, op1=ALU.add)
                    for j in range(4):
                        nc.vector.bn_stats(out=st2_all[:, g, j, :],
                                           in_=x2_all[:, g, j, :])

                # pre-initialize the constant-1 column in the two "h" ring
                # buffers (LN applies never touch column C, transposes read it)
                for _ in range(2):
                    h_init = sba.tile([128, 4, C + 1], BF16, tag="h")
                    nc.vector.memset(h_init[:, :, C:C + 1], 1.0)

                inflight = []
                for sb in range(NGROUPS // SUPER):
                    x_ts = []
                    stS = stpool.tile([128, SUPER, 4, 6], F32, tag="st")
                    scr = stpool.tile([128, SUPER * 12], F32, tag="scr")
                    var1 = vspool.tile([128, SUPER * 4], F32, tag="var1")
                    msum = vspool.tile([128, SUPER * 4], F32, tag="msum")
                    for gi in range(SUPER):
                        g = sb * SUPER + gi
                        x_t = xpool.tile([128, 4, C], F32, tag="x")
                        for wp in range(4):
                            eng = (nc.sync, nc.sync, nc.gpsimd, nc.gpsimd)[wp]
                            eng.dma_start(out=x_t[:, wp, :],
                                          in_=_pair_dram_ap(x_h, g, wp))
                        for j in range(4):
                            nc.vector.bn_stats(out=stS[:, gi, j, :],
                                               in_=x_t[:, j, :])
                        x_ts.append(x_t)
                    # one batched merge for all SUPER*4 windows
                    _merge_halves(nc, stS.tensor, stS[:].offset, stS[:].ap[0],
                                  SUPER * 4, var1[:], msum[:], scr)
                    # batched rstd = exp(-0.5 * ln(var96/96 + eps))
                    lnv = vspool.tile([128, SUPER * 4], F32, tag="lnv")
                    nc.scalar.activation(out=lnv, in_=var1[:], func=AF.Ln,
                                         bias=epsb[:], scale=1.0 / 96.0)
                    rstd1 = vspool.tile([128, SUPER * 4], F32, tag="rstd1")
                    nc.scalar.activation(out=rstd1, in_=lnv[:], func=AF.Exp,
                                         scale=-0.5)
                    # nmr = -mean*rstd = -0.5*msum*rstd (batched)
                    nmr_all = vspool.tile([128, SUPER * 4], F32, tag="nmr")
                    nc.vector.scalar_tensor_tensor(
                        out=nmr_all, in0=msum[:], scalar=-0.5,
                        in1=rstd1[:], op0=ALU.mult, op1=ALU.mult)

                    for gi in range(SUPER):
                        g = sb * SUPER + gi
                        inflight.append(do_front(g, x_ts[gi], rstd1,
                                                 nmr_all, gi))
                        if len(inflight) >= 2:
                            do_mid(inflight[-2])
                        if len(inflight) >= 3:
                            do_back1(inflight[-3])
                        if len(inflight) >= 4:
                            do_back2(inflight.pop(0))
                # pipeline flush
                do_mid(inflight[-1])
                do_back1(inflight[-2])
                do_back2(inflight.pop(0))
                do_back1(inflight[-1])
                do_back2(inflight.pop(0))
                do_back2(inflight.pop(0))

                # batched LN2 stats merge + rstd for ALL groups at once
                # (still on the exp/ln table)
                _merge_halves(nc, st2_all.tensor, st2_all[:].offset,
                              st2_all[:].ap[0], NGROUPS * 4,
                              var2_all[:], nmr2_all[:], scr2_all)
                lnv2 = cpool.tile([128, NGROUPS * 4], F32)
                nc.scalar.activation(out=lnv2, in_=var2_all[:], func=AF.Ln,
                                     bias=epsb[:], scale=1.0 / 96.0)
                nc.scalar.activation(out=rstd2_all[:], in_=lnv2[:], func=AF.Exp,
                                     scale=-0.5)
                # nmr2 = -mean*rstd = -0.5*meansum*rstd
                nc.vector.scalar_tensor_tensor(
                    out=nmr2_all[:], in0=nmr2_all[:], scalar=-0.5,
                    in1=rstd2_all[:], op0=ALU.mult, op1=ALU.mult)

            # ---------------- Phase B: MLP ----------------
            with (
                tc.tile_pool(name="sbb", bufs=2) as sbb,
                tc.tile_pool(name="psWB", bufs=2, space="PSUM") as psWB,
                tc.tile_pool(name="psG", bufs=2, space="PSUM") as psG,
            ):
                def b_front(g):
                    h2 = sbb.tile([128, 4, C], BF16, tag="h2")
                    for j in range(4):
                        nc.vector.tensor_scalar(
                            out=h2[:, j, :], in0=x2_all[:, g, j, :],
                            scalar1=rstd2_all[:, g * 4 + j:g * 4 + j + 1],
                            scalar2=nmr2_all[:, g * 4 + j:g * 4 + j + 1],
                            op0=ALU.mult, op1=ALU.add)
                    h2T_ps = psWB.tile([C, 512], BF16, tag="wb")
                    for j in range(4):
                        nc.tensor.transpose(h2T_ps[:, j * 128:(j + 1) * 128],
                                            h2[:, j, :], ident[:])
                    h2T = sbb.tile([C, 512], BF16, tag="h2T")
                    nc.vector.tensor_copy(out=h2T, in_=h2T_ps[:])
                    return dict(g=g, h2T=h2T)

                def b_mid(ctx):
                    g1_ps = psG.tile([128, 3, 512], F32, tag="g1")
                    for ch in range(3):
                        nc.tensor.matmul(g1_ps[:, ch, :],
                                         w1[:, ch * 128:(ch + 1) * 128],
                                         ctx["h2T"][:], start=True, stop=True)
                    ctx["g1_ps"] = g1_ps

                def b_gelu(ctx):
                    g1 = sbb.tile([128, 3, 512], BF16, tag="g1s")
                    nc.scalar.activation(out=g1, in_=ctx["g1_ps"][:],
                                         func=AF.Gelu)
                    ctx["g1"] = g1

                def b_back(ctx):
                    g, g1 = ctx["g"], ctx["g1"]
                    f2_ps = psWB.tile([128, 4, C], F32, tag="wb")
                    for j in range(4):
                        for ch in range(3):
                            nc.tensor.matmul(
                                f2_ps[:, j, :],
                                g1[:, ch, j * 128:(j + 1) * 128],
                                w2[:, ch, :],
                                start=(ch == 0), stop=(ch == 2),
                            )
                    out_t = sbb.tile([128, 4, C], F32, tag="outt")
                    nc.vector.scalar_tensor_tensor(
                        out=out_t[:], in0=f2_ps[:], scalar=1.0,
                        in1=x2_all[:, g], op0=ALU.mult, op1=ALU.add)
                    for wp in range(4):
                        eng = (nc.sync, nc.sync, nc.gpsimd, nc.gpsimd)[wp]
                        eng.dma_start(out=_pair_dram_ap(out_h, g, wp),
                                      in_=out_t[:, wp, :])

                bq = []
                for g in range(NGROUPS):
                    bq.append(b_front(g))
                    if len(bq) >= 2:
                        b_mid(bq[-2])
                    if len(bq) >= 3:
                        b_gelu(bq[-3])
                    if len(bq) >= 4:
                        b_back(bq.pop(0))
                b_mid(bq[-1])
                b_gelu(bq[-2])
                b_back(bq.pop(0))
                b_gelu(bq[-1])
                b_back(bq.pop(0))
                b_back(bq.pop(0))

    nc.compile()
    return nc


def _get_program():
    if "nc" not in _CACHE:
        _CACHE["nc"] = _build_program()
    return _CACHE["nc"]


def _prep_consts(norm1_g, norm1_b, qkv_w, qkv_b, proj_w, proj_b,
                 rel_bias_table, norm2_g, norm2_b, fc1_w, fc1_b, fc2_w, fc2_b):
    # Fold LN1 affine into qkv weights; fold attention scale into the q part.
    wqkv = qkv_w * norm1_g[:, None]
    bqkv = norm1_b @ qkv_w + qkv_b            # (288,)
    wqkv = wqkv.copy()
    wqkv[:, 0:C] *= SCALE
    bqkv = bqkv.copy()
    bqkv[0:C] *= SCALE
    assert np.allclose(bqkv, 0.0), "nonzero folded qkv bias not supported"
    assert np.allclose(proj_b, 0.0) and np.allclose(fc1_b, 0.0) and np.allclose(fc2_b, 0.0), \
        "nonzero proj/fc biases not supported"

    wqk = wqkv[:, 0:2 * C]
    vW = wqkv[:, 2 * C:3 * C]
    wv = np.zeros((C + 1, 99), np.float32)
    for hh in range(HEADS):
        wv[0:C, hh * 33:hh * 33 + HD] = vW[:, hh * HD:(hh + 1) * HD]
        wv[C, hh * 33 + HD] = 1.0

    w1 = fc1_w * norm2_g[:, None]
    b1 = norm2_b @ fc1_w + fc1_b
    assert np.allclose(b1, 0.0), "nonzero folded fc1 bias not supported"

    # additive bias^T blocks: logbT[q', h, k] = bias[q, k, h] when q' and k
    # are in the same window of the pair, NEG (mask) otherwise. Token order
    # on partitions is r-major: p = r*16 + par*8 + c.
    bias = rel_bias_table[REL_IDX]            # (64, 64, HEADS) [q, k, h]
    pidx = np.arange(128)
    tok = (pidx // 16) * 8 + (pidx % 8)       # token index within window
    par = (pidx // 8) % 2                     # which window of the pair
    same = par[:, None] == par[None, :]       # [q', k]
    logbT = np.where(same[:, None, :],
                     bias[tok[:, None], tok[None, :]].transpose(0, 2, 1),
                     NEG).astype(np.float32)  # [q', h, k]

    w2 = fc2_w.reshape(3, 128, C)

    return {
        "wqk": wqk,
        "wv": wv,
        "wproj": proj_w,
        "w1": w1,
        "w2": w2,
        "logbT": logbT,
    }


def _to_bf16(a):
    import ml_dtypes
    return np.asarray(a, dtype=np.float32).astype(ml_dtypes.bfloat16)


LAST_RESULTS = None


def kernel(**inputs):
    global LAST_RESULTS
    x = np.asarray(inputs["x"], np.float32)
    consts = _prep_consts(
        np.asarray(inputs["norm1_g"], np.float32), np.asarray(inputs["norm1_b"], np.float32),
        np.asarray(inputs["qkv_w"], np.float32), np.asarray(inputs["qkv_b"], np.float32),
        np.asarray(inputs["proj_w"], np.float32), np.asarray(inputs["proj_b"], np.float32),
        np.asarray(inputs["rel_bias_table"], np.float32),
        np.asarray(inputs["norm2_g"], np.float32), np.asarray(inputs["norm2_b"], np.float32),
        np.asarray(inputs["fc1_w"], np.float32), np.asarray(inputs["fc1_b"], np.float32),
        np.asarray(inputs["fc2_w"], np.float32), np.asarray(inputs["fc2_b"], np.float32),
    )

    shared = {
        "wqk": _to_bf16(consts["wqk"]),
        "wv": _to_bf16(consts["wv"]),
        "wproj": _to_bf16(consts["wproj"]),
        "w1": _to_bf16(consts["w1"]),
        "w2": _to_bf16(consts["w2"]),
        "logbT": _to_bf16(consts["logbT"]),
        "ident": _to_bf16(np.eye(128, dtype=np.float32)),
        "identrep": _to_bf16(np.tile(np.eye(128, dtype=np.float32), (1, 4))),
    }

    xr = x.reshape(B * H, W, C)
    in_maps = []
    for c in range(NCORES):
        m = dict(shared)
        m["x"] = np.ascontiguousarray(xr[c * ROWS:(c + 1) * ROWS])
        in_maps.append(m)

    nc = _get_program()
    import os
    trace = bool(os.environ.get("KERNEL_TRACE"))
    res = bass_utils.run_bass_kernel_spmd(nc, in_maps, core_ids=list(range(NCORES)),
                                          trace=trace)
    LAST_RESULTS = res
    out = np.concatenate([r["out"] for r in res.results], axis=0)
    return out.reshape(B, H, W, C)


if __name__ == "__main__":
    print("building program...")
    _get_program()
    print("program built ok")
